# revision 8
# baseline (speedup 1.0000x reference)
"""2-layer GraphConv GNN on 8 trn2 NeuronCores (Bass/Tile).

Strategy (hardcoded for N=100000 nodes, E=1600000 edges, F=128, H=128, O=64):
  - Shard edges by destination node: core c owns dst in [c*12500, (c+1)*12500).
  - Host uploads ONLY the core's x shard (bf16, 3.2MB); the full gather table
    is formed on-device via AllGather (D2D links ≫ host tunnel bandwidth).
  - Aggregation via PE matmul segment-sum: edges chunked 128 at a time;
    msgs [128 edges, 128 feat] (bf16, gathered via dma_gather) as lhsT,
    one-hot S [128 edges, 128 dst-slots] (built on DVE via iota==dst compare)
    as rhs; accumulate into PSUM [128 feat, 128 dst] per 128-dst group.
  - Gather: dma_gather (int16 idx) with sources split into 4 ranges of 25000
    rows; 4 SWDGE queues. Edges laid out in slots grouped by (supergroup,
    src-range, dst-group), dst-sorted, padded to fixed budgets (SPMD-uniform
    across cores; pad dst=200 -> zero one-hot column).
  - Gather indices uploaded as [16, cols] and broadcast to 128 partitions
    on-device (the SWDGE ucode wants 8x16-partition replication).
  - Root-term xT derived on-device from the x shard via PE transposes.
  - Inter-layer exchange: AllGather of h (bf16, 128-wide); layer 2 is all-bf16.
  - Output downloaded as bf16, cast to fp32 on host.
"""

import numpy as np
import ml_dtypes
from contextlib import ExitStack

N = 100000
F = 128          # input/hidden feature dim
O = 64           # output dim
NC = 8
SHARD = N // NC          # 12500
G = 128                  # dst nodes per psum group
NGROUP = (SHARD + G - 1) // G   # 98 (last group has 84 nodes)
LASTG = SHARD - (NGROUP - 1) * G  # 84
NR = 4                   # src ranges (int16 gather index limit)
RS = N // NR             # 25000
SB = 640                 # slot budget per (group, range); 5 chunks of 128
CHUNKS_PER_SEG = SB // 128  # 5
SG_SIZE = 4              # groups per supergroup (gather call batching)

bf16 = ml_dtypes.bfloat16


def _supergroups():
    sgs = []
    g0 = 0
    while g0 < NGROUP:
        sgs.append(list(range(g0, min(g0 + SG_SIZE, NGROUP))))
        g0 += SG_SIZE
    return sgs


SGS = _supergroups()
NCHUNKS = NGROUP * NR * CHUNKS_PER_SEG  # 1960 chunks per layer
TOTSLOTS = NGROUP * NR * SB             # 250880


def _prep_core(src, dst_local):
    """Slot layout for one core. Returns idx16 [16, TOTSLOTS//16] (int16,
    per-call 16-wrapped) and dstS [128, NCHUNKS] bf16."""
    g = dst_local // G
    r = src // RS
    bucket = g * NR + r
    order = np.lexsort((dst_local, bucket))
    s_o = src[order]
    d_o = dst_local[order]
    b_o = bucket[order]
    cnt = np.bincount(b_o, minlength=NGROUP * NR)
    if cnt.max() > SB:
        raise RuntimeError(f"bucket overflow: {cnt.max()} > {SB}")

    # slot base per bucket in (sg, r, g_local) call-major order
    slotbase = np.zeros(NGROUP * NR, dtype=np.int64)
    pos = 0
    for sg in SGS:
        for r_ in range(NR):
            for g_ in sg:
                slotbase[g_ * NR + r_] = pos
                pos += SB
    start = np.zeros(NGROUP * NR + 1, dtype=np.int64)
    np.cumsum(cnt, out=start[1:])
    within = np.arange(len(b_o)) - start[b_o]
    slot = slotbase[b_o] + within

    # pad slots gather row 0 of the range (S row is zero, so value unused).
    idx_val = np.zeros(TOTSLOTS, dtype=np.int16)
    idx_val[slot] = (s_o - (s_o // RS) * RS).astype(np.int16)
    dst_val = np.full(TOTSLOTS, 200, dtype=np.float32)  # pad: no iota match
    dst_val[slot] = (d_o % G).astype(np.float32)

    # per-call 16-wrap: call = (sg, r) covering len(sg)*SB slots
    cols = []
    pos = 0
    for sg in SGS:
        ncall = len(sg) * SB
        for r_ in range(NR):
            blk = idx_val[pos : pos + ncall]
            cols.append(blk.reshape(ncall // 16, 16).T)  # [16, ncall/16]
            pos += ncall
    idx16 = np.concatenate(cols, axis=1)  # [16, TOTSLOTS/16]

    dstS = np.ascontiguousarray(dst_val.reshape(NCHUNKS, 128).T).astype(bf16)
    return idx16, dstS


def _build_program():
    import concourse.bass as bass  # noqa: F401
    import concourse.tile as tile
    from concourse import bacc, mybir

    nc = bacc.Bacc(None, target_bir_lowering=False, num_swdge_queues=4)
    dt = mybir.dt

    # inputs (per-core)
    xs_in = nc.dram_tensor("xs", [SHARD, F], dt.bfloat16, kind="ExternalInput")
    idx16_in = nc.dram_tensor("idx16", [16, TOTSLOTS // 16], dt.int16, kind="ExternalInput")
    dstS_in = nc.dram_tensor("dstS", [128, NCHUNKS], dt.bfloat16, kind="ExternalInput")
    wr1T_in = nc.dram_tensor("wr1T", [F, F], dt.float32, kind="ExternalInput")
    wo1T_in = nc.dram_tensor("wo1T", [F, F], dt.bfloat16, kind="ExternalInput")
    wr2T_in = nc.dram_tensor("wr2T", [F, O], dt.float32, kind="ExternalInput")
    wo2T_in = nc.dram_tensor("wo2T", [F, O], dt.bfloat16, kind="ExternalInput")
    b1_in = nc.dram_tensor("b1", [1, F], dt.float32, kind="ExternalInput")
    b2_in = nc.dram_tensor("b2", [1, O], dt.float32, kind="ExternalInput")
    iota_in = nc.dram_tensor("iota", [128, G], dt.bfloat16, kind="ExternalInput")
    identb_in = nc.dram_tensor("identb", [128, 128], dt.bfloat16, kind="ExternalInput")
    ones_in = nc.dram_tensor("ones", [1, G], dt.float32, kind="ExternalInput")
    out_t = nc.dram_tensor("out", [SHARD, O], dt.bfloat16, kind="ExternalOutput")

    # internal DRAM
    xs_int = nc.dram_tensor("xs_int", [SHARD, F], dt.bfloat16)
    x_full = nc.dram_tensor("x_full", [N, F], dt.bfloat16, addr_space="Shared")
    h_shard = nc.dram_tensor("h_shard", [SHARD, F], dt.bfloat16)
    h_full = nc.dram_tensor("h_full", [N, F], dt.bfloat16, addr_space="Shared")

    with tile.TileContext(nc) as tc, ExitStack() as ctx:
        const_p = ctx.enter_context(tc.tile_pool(name="const", bufs=1))
        resid_p = ctx.enter_context(tc.tile_pool(name="resid", bufs=1))
        xn_p = ctx.enter_context(tc.tile_pool(name="xn", bufs=4))
        msgs_p = ctx.enter_context(tc.tile_pool(name="msgs", bufs=8))
        s_p = ctx.enter_context(tc.tile_pool(name="sp", bufs=8))
        agg_p = ctx.enter_context(tc.tile_pool(name="aggp", bufs=3))
        hsb_p = ctx.enter_context(tc.tile_pool(name="hsb", bufs=3))
        osb_p = ctx.enter_context(tc.tile_pool(name="osb", bufs=3))
        ps_agg = ctx.enter_context(tc.tile_pool(name="ps_agg", bufs=2, space="PSUM"))
        ps_h = ctx.enter_context(tc.tile_pool(name="ps_h", bufs=2, space="PSUM"))
        ps_t = ctx.enter_context(tc.tile_pool(name="ps_t", bufs=1, space="PSUM"))

        # form the full gather table from the 8 shards ASAP (D2D).
        # collectives can't read IO tensors, so bounce through internal DRAM.
        nc.gpsimd.dma_start(xs_int[:], xs_in[:])
        nc.gpsimd.collective_compute(
            "AllGather",
            mybir.AluOpType.bypass,
            replica_groups=[list(range(NC))],
            ins=[xs_int[:]],
            outs=[x_full[:]],
        )

        # constants / residents
        c_iota = const_p.tile([128, G], dt.bfloat16)
        nc.sync.dma_start(c_iota[:], iota_in[:])
        c_identb = const_p.tile([128, 128], dt.bfloat16)
        nc.sync.dma_start(c_identb[:], identb_in[:])
        c_ones = const_p.tile([1, G], dt.float32)
        nc.sync.dma_start(c_ones[:], ones_in[:])
        c_wr1T = const_p.tile([F, F], dt.float32)
        nc.sync.dma_start(c_wr1T[:], wr1T_in[:])
        c_wo1T = const_p.tile([F, F], dt.bfloat16)
        nc.sync.dma_start(c_wo1T[:], wo1T_in[:])
        c_wr2T = const_p.tile([F, O], dt.float32)
        nc.sync.dma_start(c_wr2T[:], wr2T_in[:])
        c_wo2T = const_p.tile([F, O], dt.bfloat16)
        nc.sync.dma_start(c_wo2T[:], wo2T_in[:])
        c_b1 = const_p.tile([1, F], dt.float32)
        nc.sync.dma_start(c_b1[:], b1_in[:])
        c_b2 = const_p.tile([1, O], dt.float32)
        nc.sync.dma_start(c_b2[:], b2_in[:])
        c_dstS_bf = const_p.tile([128, NCHUNKS], dt.bfloat16)
        nc.sync.dma_start(c_dstS_bf[:], dstS_in[:])
        c_dstS = const_p.tile([128, NCHUNKS], dt.float32)
        nc.vector.tensor_copy(c_dstS[:], c_dstS_bf[:])
        # gather indices: broadcast [16, C] -> [128, C] (8x16-partition copies)
        c_idx = const_p.tile([128, TOTSLOTS // 16], dt.int16)
        for k in range(8):
            nc.sync.dma_start(c_idx[16 * k : 16 * k + 16, :], idx16_in[:])

        r_xT = resid_p.tile([F, SHARD], dt.bfloat16)
        r_hT = resid_p.tile([F, SHARD], dt.bfloat16)

        # root-term xT: transpose the local x shard (overlaps the AllGather)
        for g_ in range(NGROUP):
            ngn = G if g_ < NGROUP - 1 else LASTG
            gbase = g_ * G
            xn = xn_p.tile([128, F], dt.bfloat16, tag="xn")
            nc.sync.dma_start(xn[:ngn, :], xs_in[gbase : gbase + ngn, :])
            pt = ps_t.tile([128, 128], dt.bfloat16, tag="pxt", space="PSUM")
            nc.tensor.transpose(pt[:F, :ngn], xn[:ngn, :F], c_identb[:ngn, :ngn])
            nc.scalar.copy(out=r_xT[:, gbase : gbase + ngn], in_=pt[:F, :ngn])

        def layer(L):
            """L=1: table=x_full, produce h (hT resident + h_shard DRAM).
            L=2: table=h_full, produce out."""
            table = x_full if L == 1 else h_full
            call_idx = 0   # column offset into c_idx (units of 16-wrapped cols)
            chunk_idx = 0  # global chunk counter (dstS column)
            for sg in SGS:
                ng = len(sg)
                call_slots = ng * SB
                call_cols = call_slots // 16
                blocks = call_slots // 128
                msgs = []
                for r_ in range(NR):
                    m = msgs_p.tile([128, blocks * F], dt.bfloat16, tag="m")
                    nc.gpsimd.dma_gather(
                        m[:].rearrange("p (c e) -> p c e", e=F),
                        table[r_ * RS : (r_ + 1) * RS, :],
                        c_idx[:, call_idx : call_idx + call_cols],
                        call_slots,
                        call_slots,
                        F,
                        single_packet=False,
                        queue_num=r_,
                    )
                    msgs.append(m)
                    call_idx += call_cols
                for gl, g_ in enumerate(sg):
                    ngn = G if g_ < NGROUP - 1 else LASTG
                    gbase = g_ * G
                    psum = ps_agg.tile([128, G], dt.float32, tag="agg", space="PSUM")
                    nmm = NR * CHUNKS_PER_SEG
                    mm = 0
                    for r_ in range(NR):
                        for k in range(CHUNKS_PER_SEG):
                            b = gl * CHUNKS_PER_SEG + k
                            # chunk index in slot layout: (sg, r, g_local, k)
                            ci = chunk_idx + (r_ * ng + gl) * CHUNKS_PER_SEG + k
                            S = s_p.tile([128, G], dt.bfloat16, tag="S")
                            nc.vector.tensor_scalar(
                                out=S[:],
                                in0=c_iota[:],
                                scalar1=c_dstS[:, ci : ci + 1],
                                scalar2=None,
                                op0=mybir.AluOpType.is_equal,
                            )
                            nc.tensor.matmul(
                                psum[:F, :],
                                lhsT=msgs[r_][:, b * F : (b + 1) * F],
                                rhs=S[:],
                                start=(mm == 0),
                                stop=(mm == nmm - 1),
                            )
                            mm += 1
                    aggT = agg_p.tile([128, G], dt.float32, tag="aggT")
                    nc.scalar.copy(out=aggT[:F, :], in_=psum[:F, :])
                    if L == 1:
                        ph = ps_h.tile([128, G], dt.float32, tag="ph", space="PSUM")
                        nc.tensor.matmul(ph[:], lhsT=c_wr1T[:], rhs=aggT[:], start=True, stop=False)
                        nc.tensor.matmul(ph[:, :ngn], lhsT=c_wo1T[:], rhs=r_xT[:, gbase : gbase + ngn], start=False, stop=False)
                        nc.tensor.matmul(ph[:, :ngn], lhsT=c_b1[:1, :], rhs=c_ones[:1, :ngn], start=False, stop=True)
                        # relu -> hT resident (bf16)
                        nc.scalar.activation(
                            out=r_hT[:, gbase : gbase + ngn],
                            in_=ph[:, :ngn],
                            func=mybir.ActivationFunctionType.Relu,
                        )
                        # transpose -> node-major h (bf16) -> DRAM for AllGather
                        pt = ps_t.tile([128, 128], dt.bfloat16, tag="pt", space="PSUM")
                        nc.tensor.transpose(pt[:ngn, :F], r_hT[:, gbase : gbase + ngn], c_identb[:, :])
                        hsb = hsb_p.tile([128, F], dt.bfloat16, tag="hsb")
                        nc.scalar.copy(out=hsb[:ngn, :], in_=pt[:ngn, :F])
                        nc.sync.dma_start(h_shard[gbase : gbase + ngn, :], hsb[:ngn, :])
                    else:
                        po = ps_h.tile([128, O], dt.float32, tag="po", space="PSUM")
                        nc.tensor.matmul(po[:ngn, :], lhsT=aggT[:, :ngn], rhs=c_wr2T[:], start=True, stop=False)
                        nc.tensor.matmul(po[:ngn, :], lhsT=r_hT[:, gbase : gbase + ngn], rhs=c_wo2T[:], start=False, stop=False)
                        nc.tensor.matmul(po[:ngn, :], lhsT=c_ones[:1, :ngn], rhs=c_b2[:1, :], start=False, stop=True)
                        osb = osb_p.tile([128, O], dt.bfloat16, tag="osb")
                        nc.scalar.copy(out=osb[:ngn, :], in_=po[:ngn, :])
                        nc.sync.dma_start(out_t[gbase : gbase + ngn, :], osb[:ngn, :])
                chunk_idx += ng * NR * CHUNKS_PER_SEG

        layer(1)
        nc.gpsimd.collective_compute(
            "AllGather",
            mybir.AluOpType.bypass,
            replica_groups=[list(range(NC))],
            ins=[h_shard[:]],
            outs=[h_full[:]],
        )
        layer(2)

    nc.finalize()
    return nc


_CACHED = {}


def prepare_in_maps(inputs):
    x = np.asarray(inputs["x"], dtype=np.float32)
    edge_index = np.asarray(inputs["edge_index"])
    w_rel1 = np.asarray(inputs["w_rel1"], dtype=np.float32)
    b_rel1 = np.asarray(inputs["b_rel1"], dtype=np.float32)
    w_root1 = np.asarray(inputs["w_root1"], dtype=np.float32)
    w_rel2 = np.asarray(inputs["w_rel2"], dtype=np.float32)
    b_rel2 = np.asarray(inputs["b_rel2"], dtype=np.float32)
    w_root2 = np.asarray(inputs["w_root2"], dtype=np.float32)

    src = edge_index[0].astype(np.int64)
    dst = edge_index[1].astype(np.int64)

    iota = np.broadcast_to(np.arange(G, dtype=np.float32), (128, G)).astype(bf16)
    identb = np.eye(128, dtype=np.float32).astype(bf16)
    ones = np.ones((1, G), dtype=np.float32)
    wr1T = np.ascontiguousarray(w_rel1.T)
    wo1T = np.ascontiguousarray(w_root1.T).astype(bf16)
    wr2T = np.ascontiguousarray(w_rel2.T)
    wo2T = np.ascontiguousarray(w_root2.T).astype(bf16)
    b1 = b_rel1.reshape(1, F)
    b2 = b_rel2.reshape(1, O)

    in_maps = []
    for c in range(NC):
        m = (dst >= c * SHARD) & (dst < (c + 1) * SHARD)
        idx16, dstS = _prep_core(src[m], dst[m] - c * SHARD)
        in_maps.append(
            {
                "xs": x[c * SHARD : (c + 1) * SHARD, :].astype(bf16),
                "idx16": idx16,
                "dstS": dstS,
                "wr1T": wr1T,
                "wo1T": wo1T,
                "wr2T": wr2T,
                "wo2T": wo2T,
                "b1": b1,
                "b2": b2,
                "iota": iota,
                "identb": identb,
                "ones": ones,
            }
        )
    return in_maps


def get_nc():
    if "nc" not in _CACHED:
        _CACHED["nc"] = _build_program()
    return _CACHED["nc"]


def kernel(**inputs):
    from concourse.bass_utils import run_bass_kernel_spmd

    in_maps = prepare_in_maps(inputs)
    nc = get_nc()
    res = run_bass_kernel_spmd(nc, in_maps, core_ids=list(range(NC)), trace=False)
    out = np.concatenate([res.results[c]["out"] for c in range(NC)], axis=0)
    return out.astype(np.float32)
